# revision 1
# baseline (speedup 1.0000x reference)
"""GraphNorm Trainium2 kernel.

Problem: GraphNorm over N=500000 nodes, C=128 channels, B=512 graphs,
`batch` sorted. out = weight * (x - mean[batch]*mean_scale) / sqrt(var[batch]+eps) + bias
with per-graph mean/var of the mean_scale-centered features.

Strategy (8 cores, SPMD):
  - Graph-aligned data-parallel sharding over nodes (no graph straddles a
    core, so no cross-core reduction at all).
  - Per core, nodes are packed into chunks of 8192 (64 tiles of 128 nodes)
    aligned to graph boundaries; each chunk holds <= 32 graphs ("slots").
  - One pass over HBM: x is loaded once per chunk into SBUF, per-slot
    sums of [x | x^2] are computed with one-hot matmuls (A^T @ [x|x^2]
    accumulated in PSUM over the chunk), stats are turned into per-slot
    affine maps W = w*istd, B = b - mean*s*W, and the output
    out = x*W[slot] + B[slot] is produced by a one-hot gather matmul
    (A @ [W|B]) plus two vector ops, then stored.
  - var uses E[(x-s*m)^2] = E[x^2] - (2s - s^2) m^2 (exact identity).
  - Matmuls run as float32r (fast PE mode). The gather is split into
    hi+lo f32r matmuls accumulated in PSUM, recovering full f32 precision
    of the per-slot affine maps. Stats sums see only the f32r rounding of
    x (~1e-4 relative per element), which averages out over ~1000-node
    graphs (absolute mean/var error ~1e-5).
"""

import sys

sys.path.insert(0, "/opt/trn_rl_repo")

import numpy as np

import concourse.bass as bass
import concourse.bacc as bacc
import concourse.tile as tile
from concourse import mybir
from concourse.bass_utils import run_bass_kernel_spmd

f32 = mybir.dt.float32
f32r = mybir.dt.float32r
i32 = mybir.dt.int32

N, C, B = 500000, 128, 512
EPS = 1e-5
NCORES = 8
TPC = 64            # tiles per chunk
CHUNK = TPC * 128   # 8192 nodes per chunk
S = 64              # graph slots per chunk
GRP = 128 // S      # tiles per stacked A block (2)
NBLK = TPC // GRP   # stacked blocks per chunk (16)

_prog_cache = {}


def _build_program(nch):
    nc = bacc.Bacc()
    xin = nc.dram_tensor("xin", [nch * CHUNK, C], f32, kind="ExternalInput")
    bT = nc.dram_tensor("bT", [nch, 128, TPC], f32, kind="ExternalInput")
    invr = nc.dram_tensor("invr", [nch, 128, 1], f32, kind="ExternalInput")
    pb = nc.dram_tensor("pb", [128, 512], f32, kind="ExternalInput")
    outp = nc.dram_tensor("outp", [nch * CHUNK, C], f32, kind="ExternalOutput")

    with tile.TileContext(nc) as tc:
        with tc.tile_pool(name="const", bufs=1) as constp, \
             tc.tile_pool(name="dpool", bufs=2) as dpool, \
             tc.tile_pool(name="opool", bufs=2) as opool, \
             tc.tile_pool(name="btp", bufs=2) as btp, \
             tc.tile_pool(name="a4p", bufs=6) as a4p, \
             tc.tile_pool(name="at4p", bufs=2 * NBLK) as at4p, \
             tc.tile_pool(name="combp", bufs=4) as combp, \
             tc.tile_pool(name="statp", bufs=4) as statp, \
             tc.tile_pool(name="wbp", bufs=4) as wbp, \
             tc.tile_pool(name="pst_pool", bufs=2, space="PSUM") as pstp, \
             tc.tile_pool(name="atp_pool", bufs=2, space="PSUM") as atpp, \
             tc.tile_pool(name="pg_pool", bufs=4, space="PSUM") as pgp:

            # constants
            iota_sf = constp.tile([128, 128], f32)
            iota_si = constp.tile([128, 128], i32)
            nc.gpsimd.iota(iota_si, pattern=[[0, GRP], [1, S]], base=0,
                           channel_multiplier=0)
            nc.vector.tensor_copy(out=iota_sf, in_=iota_si)
            ident = constp.tile([128, 128], f32r)
            identi = constp.tile([128, 128], i32)
            nc.gpsimd.iota(identi, pattern=[[-1, 128]], base=127,
                           channel_multiplier=1)
            nc.vector.tensor_scalar(out=ident, in0=identi, scalar1=127,
                                    scalar2=None, op0=mybir.AluOpType.is_equal)
            pbt = constp.tile([128, 512], f32)
            nc.sync.dma_start(out=pbt, in_=pb[:, :])
            epst = constp.tile([128, 1], f32)
            nc.vector.memset(epst, EPS)

            for c in range(nch):
                # ---- loads
                D = dpool.tile([128, CHUNK], f32, tag="D")
                nc.sync.dma_start(
                    out=D.rearrange("p (t c) -> p t c", c=C),
                    in_=xin.ap()[c * CHUNK:(c + 1) * CHUNK, :]
                        .rearrange("(t p) c -> p t c", p=128))
                bTt = btp.tile([128, TPC], f32, tag="bT")
                nc.sync.dma_start(out=bTt, in_=bT.ap()[c])
                invt = btp.tile([128, 1], f32, tag="inv")
                nc.sync.dma_start(out=invt, in_=invr.ap()[c])

                # ---- one-hot blocks and their transposes
                A4s, AT4s = [], []
                for blk in range(NBLK):
                    A4 = a4p.tile([128, 128], f32r, tag="A4")
                    in0 = bass.AP(tensor=bTt.tensor,
                                  offset=bTt.offset + blk * GRP,
                                  ap=[bTt.ap[0], [1, GRP], [0, S]])
                    nc.vector.tensor_tensor(
                        out=A4.rearrange("p (g s) -> p g s", s=S),
                        in0=in0,
                        in1=iota_sf.rearrange("p (g s) -> p g s", s=S),
                        op=mybir.AluOpType.is_equal)
                    atp = atpp.tile([128, 128], f32r, tag="atp")
                    nc.tensor.transpose(atp, A4, ident)
                    AT4 = at4p.tile([128, 128], f32r, tag="AT4")
                    nc.scalar.copy(out=AT4, in_=atp)
                    A4s.append(A4)
                    AT4s.append(AT4)

                # ---- stats accumulation over the chunk
                pst = pstp.tile([S, 256], f32, tag="pst")
                for t in range(TPC):
                    blk, g = divmod(t, GRP)
                    comb = combp.tile([128, 256], f32r, tag="comb")
                    nc.gpsimd.tensor_copy(out=comb[:, 0:C],
                                          in_=D[:, t * C:(t + 1) * C])
                    nc.scalar.square(out=comb[:, C:2 * C],
                                     in_=D[:, t * C:(t + 1) * C])
                    nc.tensor.matmul(pst, lhsT=A4s[blk][:, g * S:(g + 1) * S],
                                     rhs=comb, start=(t == 0),
                                     stop=(t == TPC - 1))

                # ---- per-slot affine maps (replicated x2 along partitions)
                ps0 = statp.tile([S, 256], f32, tag="ps0")
                nc.vector.tensor_copy(out=ps0, in_=pst)
                pstR = statp.tile([128, 256], f32, tag="pstR")
                for g in range(GRP):
                    nc.sync.dma_start(out=pstR[g * S:(g + 1) * S, :], in_=ps0)
                mean = statp.tile([128, 128], f32, tag="mean")
                nc.vector.tensor_scalar_mul(out=mean, in0=pstR[:, 0:128],
                                            scalar1=invt)
                ex2 = statp.tile([128, 128], f32, tag="ex2")
                nc.vector.tensor_scalar_mul(out=ex2, in0=pstR[:, 128:256],
                                            scalar1=invt)
                wbx = wbp.tile([128, 256], f32, tag="wbx")
                W = wbx[:, 0:128]
                Bv = wbx[:, 128:256]
                var = statp.tile([128, 128], f32, tag="var")
                nc.vector.tensor_mul(out=var, in0=mean, in1=mean)
                nc.vector.tensor_mul(out=var, in0=var, in1=pbt[:, 0:128])
                nc.vector.tensor_sub(out=var, in0=ex2, in1=var)
                std = statp.tile([128, 128], f32, tag="std")
                nc.scalar.activation(out=std, in_=var,
                                     func=mybir.ActivationFunctionType.Sqrt,
                                     bias=epst, scale=1.0)
                nc.vector.reciprocal(out=std, in_=std)
                nc.vector.tensor_mul(out=W, in0=std, in1=pbt[:, 256:384])
                nc.vector.tensor_mul(out=mean, in0=mean, in1=pbt[:, 128:256])
                nc.vector.tensor_mul(out=mean, in0=mean, in1=W)
                nc.vector.tensor_sub(out=Bv, in0=pbt[:, 384:512], in1=mean)
                wb_hi = wbp.tile([128, 256], f32r, tag="wb_hi")
                nc.vector.tensor_copy(out=wb_hi, in_=wbx)
                wb_lo = wbp.tile([128, 256], f32r, tag="wb_lo")
                nc.vector.tensor_sub(out=wb_lo, in0=wbx, in1=wb_hi.bitcast(f32))

                # ---- gather + elementwise + store
                OUT = opool.tile([128, CHUNK], f32, tag="OUT")
                for t in range(TPC):
                    blk, g = divmod(t, GRP)
                    pg = pgp.tile([128, 256], f32, tag="pg")
                    at = AT4s[blk][g * S:(g + 1) * S, :]
                    nc.tensor.matmul(pg, lhsT=at, rhs=wb_hi[g * S:(g + 1) * S, :],
                                     start=True, stop=False)
                    nc.tensor.matmul(pg, lhsT=at, rhs=wb_lo[g * S:(g + 1) * S, :],
                                     start=False, stop=True)
                    osl = OUT[:, t * C:(t + 1) * C]
                    nc.vector.tensor_mul(out=osl, in0=D[:, t * C:(t + 1) * C],
                                         in1=pg[:, 0:128])
                    nc.vector.tensor_add(out=osl, in0=osl, in1=pg[:, 128:256])
                nc.sync.dma_start(
                    out=outp.ap()[c * CHUNK:(c + 1) * CHUNK, :]
                        .rearrange("(t p) c -> p t c", p=128),
                    in_=OUT.rearrange("p (t c) -> p t c", c=C))

    nc.finalize()
    return nc


def _shard(batch_np):
    """Graph-aligned sharding + chunk packing. Returns per-core metadata."""
    cnt = np.bincount(batch_np, minlength=B).astype(np.int64)
    cum = np.cumsum(cnt)  # cum[g] = nodes in graphs 0..g
    # split graphs into NCORES node-balanced contiguous ranges
    targets = (np.arange(1, NCORES) * (N / NCORES))
    bounds = np.searchsorted(cum, targets)  # graph index where each core ends
    gb = [0] + [int(b) + 1 for b in bounds] + [B]
    cores = []
    for i in range(NCORES):
        g0, g1 = gb[i], gb[i + 1]
        # pack graphs [g0,g1) into chunks of <= CHUNK nodes, <= S graphs
        chunks = []
        cur, cur_nodes = [], 0
        for g in range(g0, g1):
            n_g = int(cnt[g])
            if n_g == 0:
                continue
            assert n_g <= CHUNK, f"graph {g} has {n_g} nodes > chunk"
            if cur_nodes + n_g > CHUNK or len(cur) >= S:
                chunks.append((cur, cur_nodes))
                cur, cur_nodes = [], 0
            cur.append(g)
            cur_nodes += n_g
        if cur:
            chunks.append((cur, cur_nodes))
        node0 = int(cum[g0 - 1]) if g0 > 0 else 0
        cores.append({"g0": g0, "g1": g1, "node0": node0, "chunks": chunks})
    return cores, cnt


def kernel(x, batch, weight, bias, mean_scale, batch_size):
    x = np.ascontiguousarray(np.asarray(x, dtype=np.float32))
    batch_np = np.asarray(batch).astype(np.int64)
    weight = np.asarray(weight, dtype=np.float32)
    bias = np.asarray(bias, dtype=np.float32)
    ms = np.asarray(mean_scale, dtype=np.float32)

    cores, cnt = _shard(batch_np)
    nch = max(len(c["chunks"]) for c in cores)

    # param block: [coef | s | w | b], each [128] replicated to 128 partitions
    coef = 2.0 * ms - ms * ms
    pb_row = np.concatenate([coef, ms, weight, bias]).astype(np.float32)
    pb_np = np.ascontiguousarray(np.broadcast_to(pb_row, (128, 512)))

    in_maps = []
    metas = []
    for core in cores:
        x_pad = np.zeros((nch * CHUNK, C), np.float32)
        bT_np = np.zeros((nch, 128, TPC), np.float32)
        inv_np = np.zeros((nch, 128, 1), np.float32)
        pos = core["node0"]
        meta = []
        for ci, (graphs, n_nodes) in enumerate(core["chunks"]):
            x_pad[ci * CHUNK: ci * CHUNK + n_nodes] = x[pos: pos + n_nodes]
            slots = np.zeros(CHUNK, np.float32)
            off = 0
            for si, g in enumerate(graphs):
                n_g = int(cnt[g])
                slots[off: off + n_g] = si
                inv_np[ci, si::S, 0] = 1.0 / max(n_g, 1)
                off += n_g
            bT_np[ci] = slots.reshape(TPC, 128).T
            meta.append((pos, n_nodes))
            pos += n_nodes
        in_maps.append({"xin": x_pad, "bT": bT_np, "invr": inv_np, "pb": pb_np})
        metas.append(meta)

    if nch not in _prog_cache:
        _prog_cache[nch] = _build_program(nch)
    nc = _prog_cache[nch]

    import os
    trace = os.environ.get("GN_TRACE", "0") == "1"
    kw = {}
    if trace:
        kw = {"trace": True, "tmpdir": os.environ.get("GN_TRACE_DIR") or None}
    res = run_bass_kernel_spmd(nc, in_maps, core_ids=list(range(NCORES)), **kw)
    global last_results
    last_results = res

    out = np.empty((N, C), np.float32)
    for i in range(NCORES):
        op = res.results[i]["outp"]
        for ci, (pos, n_nodes) in enumerate(metas[i]):
            out[pos: pos + n_nodes] = op[ci * CHUNK: ci * CHUNK + n_nodes]
    return out



# revision 4
# speedup vs baseline: 4.2167x; 4.2167x over previous
"""GraphNorm Trainium2 kernel (v2: channel-major fp16, no matmuls).

Problem: GraphNorm over N=500000 nodes, C=128 channels, B=512 graphs,
`batch` sorted. out = weight * (x - mean[batch]*ms) / sqrt(var[batch]+eps) + bias
with per-graph mean/var of the mean_scale-centered features;
var = E[x^2] - (2*ms - ms^2) * mean^2 (exact identity).

Strategy (8 cores, SPMD, one shared program):
  - Host casts x to fp16 and lays it out CHANNEL-MAJOR per core:
    [C=128 partitions, padded nodes]. Each graph ("slot") occupies a
    contiguous span padded to a multiple of 128 with zeros; slot
    lengths are uniform across cores (max over cores after a snake
    deal of size-sorted graphs), so a single program serves all cores.
  - Per chunk (<=16K nodes): one contiguous DMA load, per-slot
    sum(x) via DVE tensor_scalar+accum_out (4x fp16 mode), per-slot
    sum(x^2) via ACT Square+accum_out, tiny per-chunk stat math
    (mean/var/istd/W/B as [128, nslots] f32), then a single fused
    DVE tensor_scalar (x*W + B) per slot, one DMA store.
  - Zero tensor-engine work; zeros padding is harmless to the sums
    (counts come from the host as 1/n).
  - fp16 I/O halves HBM traffic vs f32; host up/down-casts and
    transposes (layout glue only, all arithmetic on device).
"""

import sys

sys.path.insert(0, "/opt/trn_rl_repo")

import numpy as np

import concourse.bass as bass
import concourse.bacc as bacc
import concourse.tile as tile
from concourse import mybir
from concourse.bass_utils import run_bass_kernel_spmd

f32 = mybir.dt.float32
f16 = mybir.dt.float16

N, C, B = 500000, 128, 512
EPS = 1e-5
NCORES = 8
CHUNK_MAX = 16384

_prog_cache = {}


def _plan(batch_np):
    cnt = np.bincount(batch_np, minlength=B).astype(np.int64)
    starts = np.zeros(B + 1, np.int64)
    np.cumsum(cnt, out=starts[1:])
    nz = [g for g in range(B) if cnt[g] > 0]
    order = sorted(nz, key=lambda g: (-int(cnt[g]), g))
    percore = [[] for _ in range(NCORES)]
    for i, g in enumerate(order):
        r, k = divmod(i, NCORES)
        if r % 2:
            k = NCORES - 1 - k
        percore[k].append(g)
    nslot = max(len(p) for p in percore)
    slot_len = []
    for j in range(nslot):
        m = 0
        for p in percore:
            if j < len(p):
                m = max(m, -(-int(cnt[p[j]]) // 128) * 128)
        assert m <= CHUNK_MAX, f"graph too large for chunk: {m}"
        slot_len.append(m)
    slot_off = []
    off = 0
    for L in slot_len:
        slot_off.append(off)
        off += L
    T = off
    # greedy chunk packing over consecutive slots
    chunks = []  # (first_slot, nslots, chunk_off, chunk_len)
    cur0, cur_len = 0, 0
    for j in range(nslot):
        if cur_len and cur_len + slot_len[j] > CHUNK_MAX:
            chunks.append((cur0, j - cur0, slot_off[cur0], cur_len))
            cur0, cur_len = j, 0
        cur_len += slot_len[j]
    if cur_len:
        chunks.append((cur0, nslot - cur0, slot_off[cur0], cur_len))
    return cnt, starts, percore, slot_len, slot_off, chunks, T


def _build(slot_len, slot_off, chunks, T):
    nslot = len(slot_len)
    A = mybir.AluOpType
    nc = bacc.Bacc()
    xcm = nc.dram_tensor("xcm", [128, T], f16, kind="ExternalInput")
    invr = nc.dram_tensor("invr", [128, nslot], f32, kind="ExternalInput")
    pb = nc.dram_tensor("pb", [128, 4], f32, kind="ExternalInput")
    outp = nc.dram_tensor("outp", [128, T], f16, kind="ExternalOutput")

    with tile.TileContext(nc) as tc:
        with tc.tile_pool(name="const", bufs=1) as constp, \
             tc.tile_pool(name="dpool", bufs=2) as dpool, \
             tc.tile_pool(name="opool", bufs=2) as opool, \
             tc.tile_pool(name="statp", bufs=2) as statp:

            invt = constp.tile([128, nslot], f32)
            nc.sync.dma_start(out=invt, in_=invr.ap()[:, :])
            pbt = constp.tile([128, 4], f32)
            nc.sync.dma_start(out=pbt, in_=pb.ap()[:, :])
            epst = constp.tile([128, 1], f32)
            nc.vector.memset(epst, EPS)
            w_col = pbt[:, 0:1]
            b_col = pbt[:, 1:2]
            negs_col = pbt[:, 2:3]
            coef_col = pbt[:, 3:4]

            for (s0, ns, coff, clen) in chunks:
                D = dpool.tile([128, CHUNK_MAX], f16, tag="D")
                nc.sync.dma_start(out=D[:, 0:clen],
                                  in_=xcm.ap()[:, coff:coff + clen])
                OUT = opool.tile([128, CHUNK_MAX], f16, tag="OUT")
                sums = statp.tile([128, 2 * ns], f32, tag="sums")
                sumx = sums[:, 0:ns]
                sumx2 = sums[:, ns:2 * ns]

                # per-slot sum(x): DVE identity-mult with accumulate
                for i in range(ns):
                    a = slot_off[s0 + i] - coff
                    e = a + slot_len[s0 + i]
                    nc.vector.tensor_scalar(
                        out=OUT[:, a:e], in0=D[:, a:e],
                        scalar1=1.0, scalar2=None,
                        op0=A.mult, op1=A.add,
                        accum_out=sumx[:, i:i + 1])
                # per-slot sum(x^2): ACT square with accumulate
                for i in range(ns):
                    a = slot_off[s0 + i] - coff
                    e = a + slot_len[s0 + i]
                    nc.scalar.activation(
                        out=OUT[:, a:e], in_=D[:, a:e],
                        func=mybir.ActivationFunctionType.Square,
                        accum_out=sumx2[:, i:i + 1])

                # per-chunk stats -> per-slot affine maps W, B ([128, ns] f32)
                st = statp.tile([128, 4 * ns], f32, tag="st")
                mean = st[:, 0:ns]
                ex2 = st[:, ns:2 * ns]
                t2 = st[:, 2 * ns:3 * ns]
                istd = st[:, 3 * ns:4 * ns]
                wb = statp.tile([128, 2 * ns], f32, tag="wb")
                Wt = wb[:, 0:ns]
                Bt = wb[:, ns:2 * ns]
                inv_sl = invt[:, s0:s0 + ns]

                nc.vector.tensor_tensor(out=mean, in0=sumx, in1=inv_sl,
                                        op=A.mult)
                nc.vector.tensor_tensor(out=ex2, in0=sumx2, in1=inv_sl,
                                        op=A.mult)
                nc.vector.tensor_tensor(out=t2, in0=mean, in1=mean, op=A.mult)
                nc.vector.tensor_scalar(out=t2, in0=t2, scalar1=coef_col,
                                        scalar2=None, op0=A.mult)
                nc.vector.tensor_tensor(out=t2, in0=ex2, in1=t2,
                                        op=A.subtract)
                nc.scalar.activation(out=t2, in_=t2,
                                     func=mybir.ActivationFunctionType.Sqrt,
                                     bias=epst)
                nc.vector.reciprocal(out=istd, in_=t2)
                nc.vector.tensor_scalar(out=Wt, in0=istd, scalar1=w_col,
                                        scalar2=None, op0=A.mult)
                nc.vector.tensor_tensor(out=t2, in0=mean, in1=Wt, op=A.mult)
                nc.vector.tensor_scalar(out=Bt, in0=t2, scalar1=negs_col,
                                        scalar2=b_col, op0=A.mult, op1=A.add)

                # fused apply: out = x*W[slot] + B[slot]
                for i in range(ns):
                    a = slot_off[s0 + i] - coff
                    e = a + slot_len[s0 + i]
                    nc.vector.tensor_scalar(
                        out=OUT[:, a:e], in0=D[:, a:e],
                        scalar1=Wt[:, i:i + 1], scalar2=Bt[:, i:i + 1],
                        op0=A.mult, op1=A.add)
                nc.sync.dma_start(out=outp.ap()[:, coff:coff + clen],
                                  in_=OUT[:, 0:clen])

    nc.finalize()
    return nc


def kernel(x, batch, weight, bias, mean_scale, batch_size):
    x = np.asarray(x, dtype=np.float32)
    batch_np = np.asarray(batch).astype(np.int64)
    w = np.asarray(weight, dtype=np.float32)
    b = np.asarray(bias, dtype=np.float32)
    s = np.asarray(mean_scale, dtype=np.float32)
    assert x.shape == (N, C) and int(batch_size) == B

    cnt, starts, percore, slot_len, slot_off, chunks, T = _plan(batch_np)
    nslot = len(slot_len)

    key = tuple(slot_len)
    if key not in _prog_cache:
        _prog_cache[key] = _build(slot_len, slot_off, chunks, T)
    nc = _prog_cache[key]

    x16 = x.astype(np.float16)
    pbm = np.ascontiguousarray(
        np.stack([w, b, -s, 2.0 * s - s * s], axis=1), dtype=np.float32)

    in_maps = []
    for k in range(NCORES):
        xb = np.zeros((T, C), np.float16)
        invm = np.zeros((nslot,), np.float32)
        for j, g in enumerate(percore[k]):
            a = int(starts[g])
            n = int(cnt[g])
            o = slot_off[j]
            xb[o:o + n] = x16[a:a + n]
            invm[j] = 1.0 / n
        xcm_np = np.ascontiguousarray(xb.T)
        inv128 = np.ascontiguousarray(
            np.broadcast_to(invm[None, :], (128, nslot)), dtype=np.float32)
        in_maps.append({"xcm": xcm_np, "invr": inv128, "pb": pbm})

    import os
    kw = {}
    if os.environ.get("GN_TRACE", "0") == "1":
        kw = {"trace": True,
              "tmpdir": os.environ.get("GN_TRACE_DIR") or None}
    res = run_bass_kernel_spmd(nc, in_maps, core_ids=list(range(NCORES)), **kw)
    global last_results
    last_results = res

    out = np.empty((N, C), np.float32)
    for k in range(NCORES):
        op = np.asarray(res.results[k]["outp"])  # [128, T] f16
        opT = np.ascontiguousarray(op.T)
        for j, g in enumerate(percore[k]):
            a = int(starts[g])
            n = int(cnt[g])
            o = slot_off[j]
            out[a:a + n] = opT[o:o + n]
    return out


# revision 7
# speedup vs baseline: 4.9165x; 1.1660x over previous
"""GraphNorm Trainium2 kernel (v3: channel-major fp16, fold-tree sums).

out = weight * (x - mean[batch]*ms) / sqrt(var[batch]+eps) + bias,
per-graph mean/var over nodes; var = E[x^2] - (2*ms - ms^2) * mean^2.

Strategy (8 cores, SPMD, one shared program):
  - Host casts x to fp16, lays it out CHANNEL-MAJOR per core
    [C=128 partitions, padded nodes]; each graph ("slot") is a
    contiguous span padded with zeros to a multiple of 128. Slot
    lengths are uniform across cores (max over cores after a snake
    deal of size-sorted graphs) so one program serves all 8 cores.
  - Slots are sorted by length, so each chunk consists of a few
    groups of EQUAL-length slots. Per group, sum(x) is computed by a
    fold tree: strided [128, ns, L] tensor_tensor adds halve L (2x
    fp16 DVE mode), then one tensor_reduce finishes. This avoids the
    slow 1x-rate per-slot accumulate path.
  - sum(x^2) comes from ACT Square+accum_out per slot (runs
    concurrently with the DVE folds; squares scratch into OUT which
    the apply later overwrites).
  - Apply is one fused DVE tensor_scalar (x*W + B) per slot (4x fp16).
  - No tensor-engine work; zero padding never corrupts sums.
"""

import sys

sys.path.insert(0, "/opt/trn_rl_repo")

import numpy as np

import concourse.bass as bass
import concourse.bacc as bacc
import concourse.tile as tile
from concourse import mybir
from concourse.bass_utils import run_bass_kernel_spmd

f32 = mybir.dt.float32
f16 = mybir.dt.float16

N, C, B = 500000, 128, 512
EPS = 1e-5
NCORES = 8
CHUNK_MAX = 16384
FOLD_MIN = 160  # stop folding at lengths <= this (or odd)

_prog_cache = {}


def _plan(batch_np):
    cnt = np.bincount(batch_np, minlength=B).astype(np.int64)
    starts = np.zeros(B + 1, np.int64)
    np.cumsum(cnt, out=starts[1:])
    nz = [g for g in range(B) if cnt[g] > 0]
    order = sorted(nz, key=lambda g: (-int(cnt[g]), g))
    percore = [[] for _ in range(NCORES)]
    for i, g in enumerate(order):
        r, k = divmod(i, NCORES)
        if r % 2:
            k = NCORES - 1 - k
        percore[k].append(g)
    nslot = max(len(p) for p in percore)
    slot_len = []
    for j in range(nslot):
        m = 0
        for p in percore:
            if j < len(p):
                m = max(m, -(-int(cnt[p[j]]) // 128) * 128)
        assert m <= CHUNK_MAX, f"graph too large for chunk: {m}"
        slot_len.append(m)
    # slot_len is non-increasing by construction
    slot_off = []
    off = 0
    for L in slot_len:
        slot_off.append(off)
        off += L
    T = off
    chunks = []  # (first_slot, nslots, chunk_off, chunk_len)
    cur0, cur_len = 0, 0
    for j in range(nslot):
        if cur_len and cur_len + slot_len[j] > CHUNK_MAX:
            chunks.append((cur0, j - cur0, slot_off[cur0], cur_len))
            cur0, cur_len = j, 0
        cur_len += slot_len[j]
    if cur_len:
        chunks.append((cur0, nslot - cur0, slot_off[cur0], cur_len))
    return cnt, starts, percore, slot_len, slot_off, chunks, T


def _build(slot_len, slot_off, chunks, T):
    nslot = len(slot_len)
    A = mybir.AluOpType
    nc = bacc.Bacc()
    xcm = nc.dram_tensor("xcm", [128, T], f16, kind="ExternalInput")
    invr = nc.dram_tensor("invr", [128, 2 * nslot], f32, kind="ExternalInput")
    pb = nc.dram_tensor("pb", [128, 4], f32, kind="ExternalInput")
    outp = nc.dram_tensor("outp", [128, T], f16, kind="ExternalOutput")

    with tile.TileContext(nc) as tc:
        with tc.tile_pool(name="const", bufs=1) as constp, \
             tc.tile_pool(name="dpool", bufs=2) as dpool, \
             tc.tile_pool(name="opool", bufs=2) as opool, \
             tc.tile_pool(name="scrp", bufs=1) as scrp, \
             tc.tile_pool(name="statp", bufs=2) as statp:

            invt = constp.tile([128, 2 * nslot], f32)
            nc.sync.dma_start(out=invt, in_=invr.ap()[:, :])
            pbt = constp.tile([128, 4], f32)
            nc.sync.dma_start(out=pbt, in_=pb.ap()[:, :])
            epst = constp.tile([128, 1], f32)
            nc.vector.memset(epst, EPS)
            w_col = pbt[:, 0:1]
            b_col = pbt[:, 1:2]
            negs_col = pbt[:, 2:3]
            coef_col = pbt[:, 3:4]

            SCR = scrp.tile([128, CHUNK_MAX], f16)

            for (s0, ns, coff, clen) in chunks:
                D = dpool.tile([128, CHUNK_MAX], f16, tag="D")
                nc.sync.dma_start(out=D[:, 0:clen],
                                  in_=xcm.ap()[:, coff:coff + clen])
                OUT = opool.tile([128, CHUNK_MAX], f16, tag="OUT")
                sums = statp.tile([128, 2 * ns], f32, tag="sums")
                sumx = sums[:, 0:ns]
                sumx2 = sums[:, ns:2 * ns]

                # ---- sum(x): per equal-length slot group, fold tree on DVE
                i = 0
                while i < ns:
                    L = slot_len[s0 + i]
                    j = i
                    while j < ns and slot_len[s0 + j] == L:
                        j += 1
                    gs = j - i  # group size
                    a = slot_off[s0 + i] - coff
                    src = D[:, a:a + gs * L].rearrange("p (s l) -> p s l", l=L)
                    scr_off = 0
                    Lc = L
                    while Lc > FOLD_MIN and Lc % 2 == 0:
                        h = Lc // 2
                        dst = SCR[:, scr_off:scr_off + gs * h].rearrange(
                            "p (s l) -> p s l", l=h)
                        nc.vector.tensor_tensor(
                            out=dst, in0=src[:, :, 0:h], in1=src[:, :, h:Lc],
                            op=A.add)
                        src = dst
                        scr_off += gs * h
                        Lc = h
                    nc.vector.tensor_reduce(
                        out=sumx[:, i:j], in_=src,
                        axis=mybir.AxisListType.X, op=A.add)
                    i = j

                # ---- sum(x^2): ACT Square + accumulate, per slot
                for i in range(ns):
                    a = slot_off[s0 + i] - coff
                    e = a + slot_len[s0 + i]
                    nc.scalar.activation(
                        out=OUT[:, a:e], in_=D[:, a:e],
                        func=mybir.ActivationFunctionType.Square,
                        accum_out=sumx2[:, i:i + 1])

                # ---- stats -> per-slot affine maps W, B ([128, ns] f32)
                st = statp.tile([128, 4 * ns], f32, tag="st")
                mom = st[:, 0:2 * ns]       # [mean | ex2]
                mean = st[:, 0:ns]
                ex2 = st[:, ns:2 * ns]
                t2 = st[:, 2 * ns:3 * ns]
                istd = st[:, 3 * ns:4 * ns]
                wb = statp.tile([128, 2 * ns], f32, tag="wb")
                Wt = wb[:, 0:ns]
                Bt = wb[:, ns:2 * ns]

                # [mean|ex2] = [sumx|sumx2] * [inv|inv] (invr holds inv twice)
                inv2 = invt.rearrange("p (h n) -> p h n", n=nslot)[:, :, s0:s0 + ns]
                nc.vector.tensor_tensor(
                    out=mom.rearrange("p (h n) -> p h n", n=ns),
                    in0=sums.rearrange("p (h n) -> p h n", n=ns),
                    in1=inv2, op=A.mult)
                nc.vector.tensor_tensor(out=t2, in0=mean, in1=mean, op=A.mult)
                nc.vector.tensor_scalar(out=t2, in0=t2, scalar1=coef_col,
                                        scalar2=None, op0=A.mult)
                nc.vector.tensor_tensor(out=t2, in0=ex2, in1=t2,
                                        op=A.subtract)
                nc.scalar.activation(out=t2, in_=t2,
                                     func=mybir.ActivationFunctionType.Sqrt,
                                     bias=epst)
                nc.vector.reciprocal(out=istd, in_=t2)
                nc.vector.tensor_scalar(out=Wt, in0=istd, scalar1=w_col,
                                        scalar2=None, op0=A.mult)
                nc.vector.tensor_tensor(out=t2, in0=mean, in1=Wt, op=A.mult)
                nc.vector.tensor_scalar(out=Bt, in0=t2, scalar1=negs_col,
                                        scalar2=b_col, op0=A.mult, op1=A.add)

                # ---- fused apply: out = x*W[slot] + B[slot]
                for i in range(ns):
                    a = slot_off[s0 + i] - coff
                    e = a + slot_len[s0 + i]
                    nc.vector.tensor_scalar(
                        out=OUT[:, a:e], in0=D[:, a:e],
                        scalar1=Wt[:, i:i + 1], scalar2=Bt[:, i:i + 1],
                        op0=A.mult, op1=A.add)
                nc.sync.dma_start(out=outp.ap()[:, coff:coff + clen],
                                  in_=OUT[:, 0:clen])

    nc.finalize()
    return nc


def kernel(x, batch, weight, bias, mean_scale, batch_size):
    x = np.asarray(x, dtype=np.float32)
    batch_np = np.asarray(batch).astype(np.int64)
    w = np.asarray(weight, dtype=np.float32)
    b = np.asarray(bias, dtype=np.float32)
    s = np.asarray(mean_scale, dtype=np.float32)
    assert x.shape == (N, C) and int(batch_size) == B

    cnt, starts, percore, slot_len, slot_off, chunks, T = _plan(batch_np)
    nslot = len(slot_len)

    key = tuple(slot_len)
    if key not in _prog_cache:
        _prog_cache[key] = _build(slot_len, slot_off, chunks, T)
    nc = _prog_cache[key]

    x16 = x.astype(np.float16)
    pbm = np.ascontiguousarray(
        np.stack([w, b, -s, 2.0 * s - s * s], axis=1), dtype=np.float32)

    in_maps = []
    for k in range(NCORES):
        xb = np.zeros((T, C), np.float16)
        invm = np.zeros((2 * nslot,), np.float32)
        for j, g in enumerate(percore[k]):
            a = int(starts[g])
            n = int(cnt[g])
            o = slot_off[j]
            xb[o:o + n] = x16[a:a + n]
            invm[j] = 1.0 / n
            invm[nslot + j] = 1.0 / n
        xcm_np = np.ascontiguousarray(xb.T)
        inv128 = np.ascontiguousarray(
            np.broadcast_to(invm[None, :], (128, 2 * nslot)), dtype=np.float32)
        in_maps.append({"xcm": xcm_np, "invr": inv128, "pb": pbm})

    import os
    kw = {}
    if os.environ.get("GN_TRACE", "0") == "1":
        kw = {"trace": True,
              "tmpdir": os.environ.get("GN_TRACE_DIR") or None}
    res = run_bass_kernel_spmd(nc, in_maps, core_ids=list(range(NCORES)), **kw)
    global last_results
    last_results = res

    out = np.empty((N, C), np.float32)
    for k in range(NCORES):
        op = np.asarray(res.results[k]["outp"])  # [128, T] f16
        opT = np.ascontiguousarray(op.T)
        for j, g in enumerate(percore[k]):
            a = int(starts[g])
            n = int(cnt[g])
            o = slot_off[j]
            out[a:a + n] = opT[o:o + n]
    return out


# revision 8
# speedup vs baseline: 5.7649x; 1.1725x over previous
"""GraphNorm Trainium2 kernel (v4: channel-major fp16, fold-tree sums,
software-pipelined DMA issue).

out = weight * (x - mean[batch]*ms) / sqrt(var[batch]+eps) + bias,
per-graph mean/var over nodes; var = E[x^2] - (2*ms - ms^2) * mean^2.

Strategy (8 cores, SPMD, one shared program):
  - Host casts x to fp16, lays it out CHANNEL-MAJOR per core
    [C=128 partitions, padded nodes]; each graph ("slot") is a
    contiguous span padded with zeros to a multiple of 128. Slot
    lengths are uniform across cores (max over cores after a snake
    deal of size-sorted graphs) so one program serves all 8 cores.
  - Slots are sorted by length, so each chunk consists of a few
    groups of EQUAL-length slots. Per group, sum(x) is a fold tree:
    strided [128, ns, L] tensor_tensor adds halve L (2x fp16 DVE),
    then one tensor_reduce finishes (avoids the 1x-rate per-slot
    accumulate path).
  - sum(x^2) via ACT Square+accum_out per slot, concurrent with the
    DVE folds (squares scratch into OUT, later overwritten by apply).
  - Apply is one fused DVE tensor_scalar (x*W + B) per slot (4x fp16).
  - Chunk loads are emitted PREFETCH chunks ahead of the compute so
    the in-order sync sequencer never parks a load behind a store's
    semaphore wait.
"""

import sys

sys.path.insert(0, "/opt/trn_rl_repo")

import numpy as np

import concourse.bass as bass
import concourse.bacc as bacc
import concourse.tile as tile
from concourse import mybir
from concourse.bass_utils import run_bass_kernel_spmd

f32 = mybir.dt.float32
f16 = mybir.dt.float16

N, C, B = 500000, 128, 512
EPS = 1e-5
NCORES = 8
CHUNK_MAX = 12288
FOLD_MIN = 160  # stop folding at lengths <= this (or odd)
PREFETCH = 2

_prog_cache = {}


def _plan(batch_np):
    cnt = np.bincount(batch_np, minlength=B).astype(np.int64)
    starts = np.zeros(B + 1, np.int64)
    np.cumsum(cnt, out=starts[1:])
    nz = [g for g in range(B) if cnt[g] > 0]
    order = sorted(nz, key=lambda g: (-int(cnt[g]), g))
    percore = [[] for _ in range(NCORES)]
    for i, g in enumerate(order):
        r, k = divmod(i, NCORES)
        if r % 2:
            k = NCORES - 1 - k
        percore[k].append(g)
    nslot = max(len(p) for p in percore)
    slot_len = []
    for j in range(nslot):
        m = 0
        for p in percore:
            if j < len(p):
                m = max(m, -(-int(cnt[p[j]]) // 128) * 128)
        assert m <= CHUNK_MAX, f"graph too large for chunk: {m}"
        slot_len.append(m)
    # slot_len is non-increasing by construction
    slot_off = []
    off = 0
    for L in slot_len:
        slot_off.append(off)
        off += L
    T = off
    chunks = []  # (first_slot, nslots, chunk_off, chunk_len)
    cur0, cur_len = 0, 0
    for j in range(nslot):
        if cur_len and cur_len + slot_len[j] > CHUNK_MAX:
            chunks.append((cur0, j - cur0, slot_off[cur0], cur_len))
            cur0, cur_len = j, 0
        cur_len += slot_len[j]
    if cur_len:
        chunks.append((cur0, nslot - cur0, slot_off[cur0], cur_len))
    # process the shortest chunk first for a fast pipeline fill
    orderc = sorted(range(len(chunks)), key=lambda i: chunks[i][3])
    chunks = [chunks[orderc[0]]] + [chunks[i] for i in sorted(orderc[1:])]
    return cnt, starts, percore, slot_len, slot_off, chunks, T


def _build(slot_len, slot_off, chunks, T):
    nslot = len(slot_len)
    A = mybir.AluOpType
    nc = bacc.Bacc()
    xcm = nc.dram_tensor("xcm", [128, T], f16, kind="ExternalInput")
    invr = nc.dram_tensor("invr", [128, 2 * nslot], f32, kind="ExternalInput")
    pb = nc.dram_tensor("pb", [128, 4], f32, kind="ExternalInput")
    outp = nc.dram_tensor("outp", [128, T], f16, kind="ExternalOutput")

    nchunk = len(chunks)

    with tile.TileContext(nc) as tc:
        with tc.tile_pool(name="const", bufs=1) as constp, \
             tc.tile_pool(name="dpool", bufs=PREFETCH + 1) as dpool, \
             tc.tile_pool(name="opool", bufs=3) as opool, \
             tc.tile_pool(name="scrp", bufs=1) as scrp, \
             tc.tile_pool(name="statp", bufs=3) as statp:

            invt = constp.tile([128, 2 * nslot], f32)
            nc.sync.dma_start(out=invt, in_=invr.ap()[:, :])
            pbt = constp.tile([128, 4], f32)
            nc.sync.dma_start(out=pbt, in_=pb.ap()[:, :])
            epst = constp.tile([128, 1], f32)
            nc.vector.memset(epst, EPS)
            w_col = pbt[:, 0:1]
            b_col = pbt[:, 1:2]
            negs_col = pbt[:, 2:3]
            coef_col = pbt[:, 3:4]

            SCR = scrp.tile([128, CHUNK_MAX], f16)

            Dt = [None] * nchunk

            def load(c):
                (s0, ns, coff, clen) = chunks[c]
                D = dpool.tile([128, CHUNK_MAX], f16, tag="D")
                nc.sync.dma_start(out=D[:, 0:clen],
                                  in_=xcm.ap()[:, coff:coff + clen])
                Dt[c] = D

            for c in range(min(PREFETCH + 1, nchunk)):
                load(c)

            for c, (s0, ns, coff, clen) in enumerate(chunks):
                D = Dt[c]
                OUT = opool.tile([128, CHUNK_MAX], f16, tag="OUT")
                sums = statp.tile([128, 2 * ns], f32, tag="sums")
                sumx = sums[:, 0:ns]
                sumx2 = sums[:, ns:2 * ns]

                # ---- sum(x): per equal-length slot group, fold tree on DVE
                i = 0
                while i < ns:
                    L = slot_len[s0 + i]
                    j = i
                    while j < ns and slot_len[s0 + j] == L:
                        j += 1
                    gs = j - i  # group size
                    a = slot_off[s0 + i] - coff
                    src = D[:, a:a + gs * L].rearrange("p (s l) -> p s l", l=L)
                    scr_off = 0
                    Lc = L
                    while Lc > FOLD_MIN and Lc % 2 == 0:
                        h = Lc // 2
                        dst = SCR[:, scr_off:scr_off + gs * h].rearrange(
                            "p (s l) -> p s l", l=h)
                        nc.vector.tensor_tensor(
                            out=dst, in0=src[:, :, 0:h], in1=src[:, :, h:Lc],
                            op=A.add)
                        src = dst
                        scr_off += gs * h
                        Lc = h
                    nc.vector.tensor_reduce(
                        out=sumx[:, i:j], in_=src,
                        axis=mybir.AxisListType.X, op=A.add)
                    i = j

                # ---- sum(x^2): ACT Square + accumulate, per slot
                for i in range(ns):
                    a = slot_off[s0 + i] - coff
                    e = a + slot_len[s0 + i]
                    nc.scalar.activation(
                        out=OUT[:, a:e], in_=D[:, a:e],
                        func=mybir.ActivationFunctionType.Square,
                        accum_out=sumx2[:, i:i + 1])

                # ---- stats -> per-slot affine maps W, B ([128, ns] f32)
                st = statp.tile([128, 4 * ns], f32, tag="st")
                mom = st[:, 0:2 * ns]       # [mean | ex2]
                mean = st[:, 0:ns]
                ex2 = st[:, ns:2 * ns]
                t2 = st[:, 2 * ns:3 * ns]
                istd = st[:, 3 * ns:4 * ns]
                wb = statp.tile([128, 2 * ns], f32, tag="wb")
                Wt = wb[:, 0:ns]
                Bt = wb[:, ns:2 * ns]

                # [mean|ex2] = [sumx|sumx2] * [inv|inv] (invr holds inv twice)
                inv2 = invt.rearrange("p (h n) -> p h n",
                                      n=nslot)[:, :, s0:s0 + ns]
                nc.vector.tensor_tensor(
                    out=mom.rearrange("p (h n) -> p h n", n=ns),
                    in0=sums.rearrange("p (h n) -> p h n", n=ns),
                    in1=inv2, op=A.mult)
                nc.vector.tensor_tensor(out=t2, in0=mean, in1=mean, op=A.mult)
                nc.vector.tensor_scalar(out=t2, in0=t2, scalar1=coef_col,
                                        scalar2=None, op0=A.mult)
                nc.vector.tensor_tensor(out=t2, in0=ex2, in1=t2,
                                        op=A.subtract)
                nc.scalar.activation(out=t2, in_=t2,
                                     func=mybir.ActivationFunctionType.Sqrt,
                                     bias=epst)
                nc.vector.reciprocal(out=istd, in_=t2)
                nc.vector.tensor_scalar(out=Wt, in0=istd, scalar1=w_col,
                                        scalar2=None, op0=A.mult)
                nc.vector.tensor_tensor(out=t2, in0=mean, in1=Wt, op=A.mult)
                nc.vector.tensor_scalar(out=Bt, in0=t2, scalar1=negs_col,
                                        scalar2=b_col, op0=A.mult, op1=A.add)

                # ---- fused apply: out = x*W[slot] + B[slot]
                for i in range(ns):
                    a = slot_off[s0 + i] - coff
                    e = a + slot_len[s0 + i]
                    nc.vector.tensor_scalar(
                        out=OUT[:, a:e], in0=D[:, a:e],
                        scalar1=Wt[:, i:i + 1], scalar2=Bt[:, i:i + 1],
                        op0=A.mult, op1=A.add)
                nc.sync.dma_start(out=outp.ap()[:, coff:coff + clen],
                                  in_=OUT[:, 0:clen])
                if c + PREFETCH + 1 < nchunk:
                    load(c + PREFETCH + 1)

    nc.finalize()
    return nc


def kernel(x, batch, weight, bias, mean_scale, batch_size):
    x = np.asarray(x, dtype=np.float32)
    batch_np = np.asarray(batch).astype(np.int64)
    w = np.asarray(weight, dtype=np.float32)
    b = np.asarray(bias, dtype=np.float32)
    s = np.asarray(mean_scale, dtype=np.float32)
    assert x.shape == (N, C) and int(batch_size) == B

    cnt, starts, percore, slot_len, slot_off, chunks, T = _plan(batch_np)
    nslot = len(slot_len)

    key = tuple(slot_len)
    if key not in _prog_cache:
        _prog_cache[key] = _build(slot_len, slot_off, chunks, T)
    nc = _prog_cache[key]

    x16 = x.astype(np.float16)
    pbm = np.ascontiguousarray(
        np.stack([w, b, -s, 2.0 * s - s * s], axis=1), dtype=np.float32)

    in_maps = []
    for k in range(NCORES):
        xb = np.zeros((T, C), np.float16)
        invm = np.zeros((2 * nslot,), np.float32)
        for j, g in enumerate(percore[k]):
            a = int(starts[g])
            n = int(cnt[g])
            o = slot_off[j]
            xb[o:o + n] = x16[a:a + n]
            invm[j] = 1.0 / n
            invm[nslot + j] = 1.0 / n
        xcm_np = np.ascontiguousarray(xb.T)
        inv128 = np.ascontiguousarray(
            np.broadcast_to(invm[None, :], (128, 2 * nslot)), dtype=np.float32)
        in_maps.append({"xcm": xcm_np, "invr": inv128, "pb": pbm})

    import os
    kw = {}
    if os.environ.get("GN_TRACE", "0") == "1":
        kw = {"trace": True,
              "tmpdir": os.environ.get("GN_TRACE_DIR") or None}
    res = run_bass_kernel_spmd(nc, in_maps, core_ids=list(range(NCORES)), **kw)
    global last_results
    last_results = res

    out = np.empty((N, C), np.float32)
    for k in range(NCORES):
        op = np.asarray(res.results[k]["outp"])  # [128, T] f16
        opT = np.ascontiguousarray(op.T)
        for j, g in enumerate(percore[k]):
            a = int(starts[g])
            n = int(cnt[g])
            o = slot_off[j]
            out[a:a + n] = opT[o:o + n]
    return out


# revision 10
# speedup vs baseline: 5.9608x; 1.0340x over previous
"""GraphNorm Trainium2 kernel (v4: channel-major fp16, fold-tree sums,
software-pipelined DMA issue).

out = weight * (x - mean[batch]*ms) / sqrt(var[batch]+eps) + bias,
per-graph mean/var over nodes; var = E[x^2] - (2*ms - ms^2) * mean^2.

Strategy (8 cores, SPMD, one shared program):
  - Host casts x to fp16, lays it out CHANNEL-MAJOR per core
    [C=128 partitions, padded nodes]; each graph ("slot") is a
    contiguous span padded with zeros to a multiple of 128. Slot
    lengths are uniform across cores (max over cores after a snake
    deal of size-sorted graphs) so one program serves all 8 cores.
  - Slots are sorted by length, so each chunk consists of a few
    groups of EQUAL-length slots. Per group, sum(x) is a fold tree:
    strided [128, ns, L] tensor_tensor adds halve L (2x fp16 DVE),
    then one tensor_reduce finishes (avoids the 1x-rate per-slot
    accumulate path).
  - sum(x^2) via ACT Square+accum_out per slot, concurrent with the
    DVE folds (squares scratch into OUT, later overwritten by apply).
  - Apply is one fused DVE tensor_scalar (x*W + B) per slot (4x fp16).
  - Chunk loads are emitted PREFETCH chunks ahead of the compute so
    the in-order sync sequencer never parks a load behind a store's
    semaphore wait.
"""

import sys

sys.path.insert(0, "/opt/trn_rl_repo")

import numpy as np

import concourse.bass as bass
import concourse.bacc as bacc
import concourse.tile as tile
from concourse import mybir
from concourse.bass_utils import run_bass_kernel_spmd

f32 = mybir.dt.float32
f16 = mybir.dt.float16

N, C, B = 500000, 128, 512
EPS = 1e-5
NCORES = 8
CHUNK_MAX = 12288
FIRST_CAP = 4096   # small first chunk -> fast pipeline fill
FOLD_MIN = 160  # stop folding at lengths <= this (or odd)
PREFETCH = 2

_prog_cache = {}


def _plan(batch_np):
    cnt = np.bincount(batch_np, minlength=B).astype(np.int64)
    starts = np.zeros(B + 1, np.int64)
    np.cumsum(cnt, out=starts[1:])
    nz = [g for g in range(B) if cnt[g] > 0]
    order = sorted(nz, key=lambda g: (-int(cnt[g]), g))
    percore = [[] for _ in range(NCORES)]
    for i, g in enumerate(order):
        r, k = divmod(i, NCORES)
        if r % 2:
            k = NCORES - 1 - k
        percore[k].append(g)
    nslot = max(len(p) for p in percore)
    slot_len = []
    for j in range(nslot):
        m = 0
        for p in percore:
            if j < len(p):
                m = max(m, -(-int(cnt[p[j]]) // 128) * 128)
        assert m <= CHUNK_MAX, f"graph too large for chunk: {m}"
        slot_len.append(m)
    # slot_len is non-increasing by construction
    slot_off = []
    off = 0
    for L in slot_len:
        slot_off.append(off)
        off += L
    T = off
    chunks = []  # (first_slot, nslots, chunk_off, chunk_len)
    cur0, cur_len = 0, 0
    for j in range(nslot):
        # keep the first chunk small: it gates pipeline fill
        cap = FIRST_CAP if not chunks else CHUNK_MAX
        if cur_len and cur_len + slot_len[j] > cap:
            chunks.append((cur0, j - cur0, slot_off[cur0], cur_len))
            cur0, cur_len = j, 0
        cur_len += slot_len[j]
    if cur_len:
        chunks.append((cur0, nslot - cur0, slot_off[cur0], cur_len))
    return cnt, starts, percore, slot_len, slot_off, chunks, T


def _build(slot_len, slot_off, chunks, T):
    nslot = len(slot_len)
    A = mybir.AluOpType
    nc = bacc.Bacc()
    xcm = nc.dram_tensor("xcm", [128, T], f16, kind="ExternalInput")
    invr = nc.dram_tensor("invr", [128, 2 * nslot], f32, kind="ExternalInput")
    pb = nc.dram_tensor("pb", [128, 4], f32, kind="ExternalInput")
    outp = nc.dram_tensor("outp", [128, T], f16, kind="ExternalOutput")

    nchunk = len(chunks)

    with tile.TileContext(nc) as tc:
        with tc.tile_pool(name="const", bufs=1) as constp, \
             tc.tile_pool(name="dpool", bufs=PREFETCH + 1) as dpool, \
             tc.tile_pool(name="opool", bufs=3) as opool, \
             tc.tile_pool(name="scrp", bufs=1) as scrp, \
             tc.tile_pool(name="statp", bufs=3) as statp:

            invt = constp.tile([128, 2 * nslot], f32)
            nc.sync.dma_start(out=invt, in_=invr.ap()[:, :])
            pbt = constp.tile([128, 4], f32)
            nc.sync.dma_start(out=pbt, in_=pb.ap()[:, :])
            epst = constp.tile([128, 1], f32)
            nc.vector.memset(epst, EPS)
            w_col = pbt[:, 0:1]
            b_col = pbt[:, 1:2]
            negs_col = pbt[:, 2:3]
            coef_col = pbt[:, 3:4]

            SCR = scrp.tile([128, CHUNK_MAX], f16)

            Dt = [None] * nchunk

            def load(c):
                (s0, ns, coff, clen) = chunks[c]
                D = dpool.tile([128, CHUNK_MAX], f16, tag="D")
                nc.sync.dma_start(out=D[:, 0:clen],
                                  in_=xcm.ap()[:, coff:coff + clen])
                Dt[c] = D

            for c in range(min(PREFETCH + 1, nchunk)):
                load(c)

            for c, (s0, ns, coff, clen) in enumerate(chunks):
                D = Dt[c]
                OUT = opool.tile([128, CHUNK_MAX], f16, tag="OUT")
                sums = statp.tile([128, 2 * ns], f32, tag="sums")
                sumx = sums[:, 0:ns]
                sumx2 = sums[:, ns:2 * ns]

                # ---- sum(x): per equal-length slot group, fold tree on DVE
                i = 0
                while i < ns:
                    L = slot_len[s0 + i]
                    j = i
                    while j < ns and slot_len[s0 + j] == L:
                        j += 1
                    gs = j - i  # group size
                    a = slot_off[s0 + i] - coff
                    src = D[:, a:a + gs * L].rearrange("p (s l) -> p s l", l=L)
                    scr_off = 0
                    Lc = L
                    while Lc > FOLD_MIN and Lc % 2 == 0:
                        h = Lc // 2
                        dst = SCR[:, scr_off:scr_off + gs * h].rearrange(
                            "p (s l) -> p s l", l=h)
                        nc.vector.tensor_tensor(
                            out=dst, in0=src[:, :, 0:h], in1=src[:, :, h:Lc],
                            op=A.add)
                        src = dst
                        scr_off += gs * h
                        Lc = h
                    nc.vector.tensor_reduce(
                        out=sumx[:, i:j], in_=src,
                        axis=mybir.AxisListType.X, op=A.add)
                    i = j

                # ---- sum(x^2): ACT Square + accumulate, per slot
                for i in range(ns):
                    a = slot_off[s0 + i] - coff
                    e = a + slot_len[s0 + i]
                    nc.scalar.activation(
                        out=OUT[:, a:e], in_=D[:, a:e],
                        func=mybir.ActivationFunctionType.Square,
                        accum_out=sumx2[:, i:i + 1])

                # ---- stats -> per-slot affine maps W, B ([128, ns] f32)
                st = statp.tile([128, 4 * ns], f32, tag="st")
                mom = st[:, 0:2 * ns]       # [mean | ex2]
                mean = st[:, 0:ns]
                ex2 = st[:, ns:2 * ns]
                t2 = st[:, 2 * ns:3 * ns]
                istd = st[:, 3 * ns:4 * ns]
                wb = statp.tile([128, 2 * ns], f32, tag="wb")
                Wt = wb[:, 0:ns]
                Bt = wb[:, ns:2 * ns]

                # [mean|ex2] = [sumx|sumx2] * [inv|inv] (invr holds inv twice)
                inv2 = invt.rearrange("p (h n) -> p h n",
                                      n=nslot)[:, :, s0:s0 + ns]
                nc.vector.tensor_tensor(
                    out=mom.rearrange("p (h n) -> p h n", n=ns),
                    in0=sums.rearrange("p (h n) -> p h n", n=ns),
                    in1=inv2, op=A.mult)
                nc.vector.tensor_tensor(out=t2, in0=mean, in1=mean, op=A.mult)
                nc.vector.tensor_scalar(out=t2, in0=t2, scalar1=coef_col,
                                        scalar2=None, op0=A.mult)
                nc.vector.tensor_tensor(out=t2, in0=ex2, in1=t2,
                                        op=A.subtract)
                nc.scalar.activation(out=t2, in_=t2,
                                     func=mybir.ActivationFunctionType.Sqrt,
                                     bias=epst)
                nc.vector.reciprocal(out=istd, in_=t2)
                nc.vector.tensor_scalar(out=Wt, in0=istd, scalar1=w_col,
                                        scalar2=None, op0=A.mult)
                nc.vector.tensor_tensor(out=t2, in0=mean, in1=Wt, op=A.mult)
                nc.vector.tensor_scalar(out=Bt, in0=t2, scalar1=negs_col,
                                        scalar2=b_col, op0=A.mult, op1=A.add)

                # ---- fused apply: out = x*W[slot] + B[slot]
                for i in range(ns):
                    a = slot_off[s0 + i] - coff
                    e = a + slot_len[s0 + i]
                    nc.vector.tensor_scalar(
                        out=OUT[:, a:e], in0=D[:, a:e],
                        scalar1=Wt[:, i:i + 1], scalar2=Bt[:, i:i + 1],
                        op0=A.mult, op1=A.add)
                nc.sync.dma_start(out=outp.ap()[:, coff:coff + clen],
                                  in_=OUT[:, 0:clen])
                if c + PREFETCH + 1 < nchunk:
                    load(c + PREFETCH + 1)

    nc.finalize()
    return nc


def kernel(x, batch, weight, bias, mean_scale, batch_size):
    x = np.asarray(x, dtype=np.float32)
    batch_np = np.asarray(batch).astype(np.int64)
    w = np.asarray(weight, dtype=np.float32)
    b = np.asarray(bias, dtype=np.float32)
    s = np.asarray(mean_scale, dtype=np.float32)
    assert x.shape == (N, C) and int(batch_size) == B

    cnt, starts, percore, slot_len, slot_off, chunks, T = _plan(batch_np)
    nslot = len(slot_len)

    key = tuple(slot_len)
    if key not in _prog_cache:
        _prog_cache[key] = _build(slot_len, slot_off, chunks, T)
    nc = _prog_cache[key]

    x16 = x.astype(np.float16)
    pbm = np.ascontiguousarray(
        np.stack([w, b, -s, 2.0 * s - s * s], axis=1), dtype=np.float32)

    in_maps = []
    for k in range(NCORES):
        xb = np.zeros((T, C), np.float16)
        invm = np.zeros((2 * nslot,), np.float32)
        for j, g in enumerate(percore[k]):
            a = int(starts[g])
            n = int(cnt[g])
            o = slot_off[j]
            xb[o:o + n] = x16[a:a + n]
            invm[j] = 1.0 / n
            invm[nslot + j] = 1.0 / n
        xcm_np = np.ascontiguousarray(xb.T)
        inv128 = np.ascontiguousarray(
            np.broadcast_to(invm[None, :], (128, 2 * nslot)), dtype=np.float32)
        in_maps.append({"xcm": xcm_np, "invr": inv128, "pb": pbm})

    import os
    kw = {}
    if os.environ.get("GN_TRACE", "0") == "1":
        kw = {"trace": True,
              "tmpdir": os.environ.get("GN_TRACE_DIR") or None}
    res = run_bass_kernel_spmd(nc, in_maps, core_ids=list(range(NCORES)), **kw)
    global last_results
    last_results = res

    out = np.empty((N, C), np.float32)
    for k in range(NCORES):
        op = np.asarray(res.results[k]["outp"])  # [128, T] f16
        opT = np.ascontiguousarray(op.T)
        for j, g in enumerate(percore[k]):
            a = int(starts[g])
            n = int(cnt[g])
            o = slot_off[j]
            out[a:a + n] = opT[o:o + n]
    return out


# revision 11
# speedup vs baseline: 6.1099x; 1.0250x over previous
"""GraphNorm Trainium2 kernel (v4: channel-major fp16, fold-tree sums,
software-pipelined DMA issue).

out = weight * (x - mean[batch]*ms) / sqrt(var[batch]+eps) + bias,
per-graph mean/var over nodes; var = E[x^2] - (2*ms - ms^2) * mean^2.

Strategy (8 cores, SPMD, one shared program):
  - Host casts x to fp16, lays it out CHANNEL-MAJOR per core
    [C=128 partitions, padded nodes]; each graph ("slot") is a
    contiguous span padded with zeros to a multiple of 128. Slot
    lengths are uniform across cores (max over cores after a snake
    deal of size-sorted graphs) so one program serves all 8 cores.
  - Slots are sorted by length, so each chunk consists of a few
    groups of EQUAL-length slots. Per group, sum(x) is a fold tree:
    strided [128, ns, L] tensor_tensor adds halve L (2x fp16 DVE),
    then one tensor_reduce finishes (avoids the 1x-rate per-slot
    accumulate path).
  - sum(x^2) via ACT Square+accum_out per slot, concurrent with the
    DVE folds (squares scratch into OUT, later overwritten by apply).
  - Apply is one fused DVE tensor_scalar (x*W + B) per slot (4x fp16).
  - Chunk loads are emitted PREFETCH chunks ahead of the compute so
    the in-order sync sequencer never parks a load behind a store's
    semaphore wait.
"""

import sys

sys.path.insert(0, "/opt/trn_rl_repo")

import numpy as np

import concourse.bass as bass
import concourse.bacc as bacc
import concourse.tile as tile
from concourse import mybir
from concourse.bass_utils import run_bass_kernel_spmd

f32 = mybir.dt.float32
f16 = mybir.dt.float16

N, C, B = 500000, 128, 512
EPS = 1e-5
NCORES = 8
CHUNK_MAX = 11264
FIRST_CAP = 4096   # small first chunk -> fast pipeline fill
FOLD_MIN = 96   # stop folding at lengths <= this (or odd)
PREFETCH = 3

_prog_cache = {}


def _plan(batch_np):
    cnt = np.bincount(batch_np, minlength=B).astype(np.int64)
    starts = np.zeros(B + 1, np.int64)
    np.cumsum(cnt, out=starts[1:])
    nz = [g for g in range(B) if cnt[g] > 0]
    order = sorted(nz, key=lambda g: (-int(cnt[g]), g))
    percore = [[] for _ in range(NCORES)]
    for i, g in enumerate(order):
        r, k = divmod(i, NCORES)
        if r % 2:
            k = NCORES - 1 - k
        percore[k].append(g)
    nslot = max(len(p) for p in percore)
    slot_len = []
    for j in range(nslot):
        m = 0
        for p in percore:
            if j < len(p):
                m = max(m, -(-int(cnt[p[j]]) // 128) * 128)
        assert m <= CHUNK_MAX, f"graph too large for chunk: {m}"
        slot_len.append(m)
    # slot_len is non-increasing by construction
    slot_off = []
    off = 0
    for L in slot_len:
        slot_off.append(off)
        off += L
    T = off
    chunks = []  # (first_slot, nslots, chunk_off, chunk_len)
    cur0, cur_len = 0, 0
    for j in range(nslot):
        # keep the first chunk small: it gates pipeline fill
        cap = FIRST_CAP if not chunks else CHUNK_MAX
        if cur_len and cur_len + slot_len[j] > cap:
            chunks.append((cur0, j - cur0, slot_off[cur0], cur_len))
            cur0, cur_len = j, 0
        cur_len += slot_len[j]
    if cur_len:
        chunks.append((cur0, nslot - cur0, slot_off[cur0], cur_len))
    return cnt, starts, percore, slot_len, slot_off, chunks, T


def _build(slot_len, slot_off, chunks, T):
    nslot = len(slot_len)
    A = mybir.AluOpType
    nc = bacc.Bacc()
    xcm = nc.dram_tensor("xcm", [128, T], f16, kind="ExternalInput")
    invr = nc.dram_tensor("invr", [128, 2 * nslot], f32, kind="ExternalInput")
    pb = nc.dram_tensor("pb", [128, 4], f32, kind="ExternalInput")
    outp = nc.dram_tensor("outp", [128, T], f16, kind="ExternalOutput")

    nchunk = len(chunks)

    with tile.TileContext(nc) as tc:
        with tc.tile_pool(name="const", bufs=1) as constp, \
             tc.tile_pool(name="dpool", bufs=PREFETCH + 1) as dpool, \
             tc.tile_pool(name="opool", bufs=3) as opool, \
             tc.tile_pool(name="scrp", bufs=1) as scrp, \
             tc.tile_pool(name="statp", bufs=3) as statp:

            invt = constp.tile([128, 2 * nslot], f32)
            nc.sync.dma_start(out=invt, in_=invr.ap()[:, :])
            pbt = constp.tile([128, 4], f32)
            nc.sync.dma_start(out=pbt, in_=pb.ap()[:, :])
            epst = constp.tile([128, 1], f32)
            nc.vector.memset(epst, EPS)
            w_col = pbt[:, 0:1]
            b_col = pbt[:, 1:2]
            negs_col = pbt[:, 2:3]
            coef_col = pbt[:, 3:4]

            SCR = scrp.tile([128, CHUNK_MAX], f16)

            Dt = [None] * nchunk

            def load(c):
                (s0, ns, coff, clen) = chunks[c]
                D = dpool.tile([128, CHUNK_MAX], f16, tag="D")
                nc.sync.dma_start(out=D[:, 0:clen],
                                  in_=xcm.ap()[:, coff:coff + clen])
                Dt[c] = D

            for c in range(min(PREFETCH + 1, nchunk)):
                load(c)

            for c, (s0, ns, coff, clen) in enumerate(chunks):
                D = Dt[c]
                OUT = opool.tile([128, CHUNK_MAX], f16, tag="OUT")
                sums = statp.tile([128, 2 * ns], f32, tag="sums")
                sumx = sums[:, 0:ns]
                sumx2 = sums[:, ns:2 * ns]

                # ---- sum(x): per equal-length slot group, fold tree on DVE
                i = 0
                while i < ns:
                    L = slot_len[s0 + i]
                    j = i
                    while j < ns and slot_len[s0 + j] == L:
                        j += 1
                    gs = j - i  # group size
                    a = slot_off[s0 + i] - coff
                    src = D[:, a:a + gs * L].rearrange("p (s l) -> p s l", l=L)
                    scr_off = 0
                    Lc = L
                    while Lc > FOLD_MIN and Lc % 2 == 0:
                        h = Lc // 2
                        dst = SCR[:, scr_off:scr_off + gs * h].rearrange(
                            "p (s l) -> p s l", l=h)
                        nc.vector.tensor_tensor(
                            out=dst, in0=src[:, :, 0:h], in1=src[:, :, h:Lc],
                            op=A.add)
                        src = dst
                        scr_off += gs * h
                        Lc = h
                    nc.vector.tensor_reduce(
                        out=sumx[:, i:j], in_=src,
                        axis=mybir.AxisListType.X, op=A.add)
                    i = j

                # ---- sum(x^2): ACT Square + accumulate, per slot
                for i in range(ns):
                    a = slot_off[s0 + i] - coff
                    e = a + slot_len[s0 + i]
                    nc.scalar.activation(
                        out=OUT[:, a:e], in_=D[:, a:e],
                        func=mybir.ActivationFunctionType.Square,
                        accum_out=sumx2[:, i:i + 1])

                # ---- stats -> per-slot affine maps W, B ([128, ns] f32)
                st = statp.tile([128, 4 * ns], f32, tag="st")
                mom = st[:, 0:2 * ns]       # [mean | ex2]
                mean = st[:, 0:ns]
                ex2 = st[:, ns:2 * ns]
                t2 = st[:, 2 * ns:3 * ns]
                istd = st[:, 3 * ns:4 * ns]
                wb = statp.tile([128, 2 * ns], f32, tag="wb")
                Wt = wb[:, 0:ns]
                Bt = wb[:, ns:2 * ns]

                # [mean|ex2] = [sumx|sumx2] * [inv|inv] (invr holds inv twice)
                inv2 = invt.rearrange("p (h n) -> p h n",
                                      n=nslot)[:, :, s0:s0 + ns]
                nc.vector.tensor_tensor(
                    out=mom.rearrange("p (h n) -> p h n", n=ns),
                    in0=sums.rearrange("p (h n) -> p h n", n=ns),
                    in1=inv2, op=A.mult)
                nc.vector.tensor_tensor(out=t2, in0=mean, in1=mean, op=A.mult)
                nc.vector.tensor_scalar(out=t2, in0=t2, scalar1=coef_col,
                                        scalar2=None, op0=A.mult)
                nc.vector.tensor_tensor(out=t2, in0=ex2, in1=t2,
                                        op=A.subtract)
                nc.scalar.activation(out=t2, in_=t2,
                                     func=mybir.ActivationFunctionType.Sqrt,
                                     bias=epst)
                nc.vector.reciprocal(out=istd, in_=t2)
                nc.vector.tensor_scalar(out=Wt, in0=istd, scalar1=w_col,
                                        scalar2=None, op0=A.mult)
                nc.vector.tensor_tensor(out=t2, in0=mean, in1=Wt, op=A.mult)
                nc.vector.tensor_scalar(out=Bt, in0=t2, scalar1=negs_col,
                                        scalar2=b_col, op0=A.mult, op1=A.add)

                # ---- fused apply: out = x*W[slot] + B[slot]
                for i in range(ns):
                    a = slot_off[s0 + i] - coff
                    e = a + slot_len[s0 + i]
                    nc.vector.tensor_scalar(
                        out=OUT[:, a:e], in0=D[:, a:e],
                        scalar1=Wt[:, i:i + 1], scalar2=Bt[:, i:i + 1],
                        op0=A.mult, op1=A.add)
                nc.sync.dma_start(out=outp.ap()[:, coff:coff + clen],
                                  in_=OUT[:, 0:clen])
                if c + PREFETCH + 1 < nchunk:
                    load(c + PREFETCH + 1)

    nc.finalize()
    return nc


def kernel(x, batch, weight, bias, mean_scale, batch_size):
    x = np.asarray(x, dtype=np.float32)
    batch_np = np.asarray(batch).astype(np.int64)
    w = np.asarray(weight, dtype=np.float32)
    b = np.asarray(bias, dtype=np.float32)
    s = np.asarray(mean_scale, dtype=np.float32)
    assert x.shape == (N, C) and int(batch_size) == B

    cnt, starts, percore, slot_len, slot_off, chunks, T = _plan(batch_np)
    nslot = len(slot_len)

    key = tuple(slot_len)
    if key not in _prog_cache:
        _prog_cache[key] = _build(slot_len, slot_off, chunks, T)
    nc = _prog_cache[key]

    x16 = x.astype(np.float16)
    pbm = np.ascontiguousarray(
        np.stack([w, b, -s, 2.0 * s - s * s], axis=1), dtype=np.float32)

    in_maps = []
    for k in range(NCORES):
        xb = np.zeros((T, C), np.float16)
        invm = np.zeros((2 * nslot,), np.float32)
        for j, g in enumerate(percore[k]):
            a = int(starts[g])
            n = int(cnt[g])
            o = slot_off[j]
            xb[o:o + n] = x16[a:a + n]
            invm[j] = 1.0 / n
            invm[nslot + j] = 1.0 / n
        xcm_np = np.ascontiguousarray(xb.T)
        inv128 = np.ascontiguousarray(
            np.broadcast_to(invm[None, :], (128, 2 * nslot)), dtype=np.float32)
        in_maps.append({"xcm": xcm_np, "invr": inv128, "pb": pbm})

    import os
    kw = {}
    if os.environ.get("GN_TRACE", "0") == "1":
        kw = {"trace": True,
              "tmpdir": os.environ.get("GN_TRACE_DIR") or None}
    res = run_bass_kernel_spmd(nc, in_maps, core_ids=list(range(NCORES)), **kw)
    global last_results
    last_results = res

    out = np.empty((N, C), np.float32)
    for k in range(NCORES):
        op = np.asarray(res.results[k]["outp"])  # [128, T] f16
        opT = np.ascontiguousarray(op.T)
        for j, g in enumerate(percore[k]):
            a = int(starts[g])
            n = int(cnt[g])
            o = slot_off[j]
            out[a:a + n] = opT[o:o + n]
    return out


# revision 22
# speedup vs baseline: 6.2395x; 1.0212x over previous
"""GraphNorm Trainium2 kernel (v4: channel-major fp16, fold-tree sums,
software-pipelined DMA issue).

out = weight * (x - mean[batch]*ms) / sqrt(var[batch]+eps) + bias,
per-graph mean/var over nodes; var = E[x^2] - (2*ms - ms^2) * mean^2.

Strategy (8 cores, SPMD, one shared program):
  - Host casts x to fp16, lays it out CHANNEL-MAJOR per core
    [C=128 partitions, padded nodes]; each graph ("slot") is a
    contiguous span padded with zeros to a multiple of 128. Slot
    lengths are uniform across cores (max over cores after a snake
    deal of size-sorted graphs) so one program serves all 8 cores.
  - Slots are sorted by length, so each chunk consists of a few
    groups of EQUAL-length slots. Per group, sum(x) is a fold tree:
    strided [128, ns, L] tensor_tensor adds halve L (2x fp16 DVE),
    then one tensor_reduce finishes (avoids the 1x-rate per-slot
    accumulate path).
  - sum(x^2) via ACT Square+accum_out per slot, concurrent with the
    DVE folds (squares scratch into OUT, later overwritten by apply).
  - Apply is one fused DVE tensor_scalar (x*W + B) per slot (4x fp16).
  - Chunk loads are emitted PREFETCH chunks ahead of the compute so
    the in-order sync sequencer never parks a load behind a store's
    semaphore wait.
"""

import sys

sys.path.insert(0, "/opt/trn_rl_repo")

import numpy as np

import concourse.bass as bass
import concourse.bacc as bacc
import concourse.tile as tile
from concourse import mybir
from concourse.bass_utils import run_bass_kernel_spmd

f32 = mybir.dt.float32
f16 = mybir.dt.float16

N, C, B = 500000, 128, 512
EPS = 1e-5
NCORES = 8
CHUNK_MAX = 13312
FIRST_CAP = 4096   # small first chunk -> fast pipeline fill
LAST_CAP = 3072    # small last chunk -> fast pipeline drain
FOLD_MIN = 96   # stop folding at lengths <= this (or odd)
PREFETCH = 2

_prog_cache = {}


def _plan(batch_np):
    cnt = np.bincount(batch_np, minlength=B).astype(np.int64)
    starts = np.zeros(B + 1, np.int64)
    np.cumsum(cnt, out=starts[1:])
    nz = [g for g in range(B) if cnt[g] > 0]
    order = sorted(nz, key=lambda g: (-int(cnt[g]), g))
    percore = [[] for _ in range(NCORES)]
    for i, g in enumerate(order):
        r, k = divmod(i, NCORES)
        if r % 2:
            k = NCORES - 1 - k
        percore[k].append(g)
    nslot = max(len(p) for p in percore)
    slot_len = []
    for j in range(nslot):
        m = 0
        for p in percore:
            if j < len(p):
                m = max(m, -(-int(cnt[p[j]]) // 128) * 128)
        assert m <= CHUNK_MAX, f"graph too large for chunk: {m}"
        slot_len.append(m)
    # slot_len is non-increasing by construction
    slot_off = []
    off = 0
    for L in slot_len:
        slot_off.append(off)
        off += L
    T = off
    chunks = []  # (first_slot, nslots, chunk_off, chunk_len)
    cur0, cur_len = 0, 0
    for j in range(nslot):
        # keep the first chunk small: it gates pipeline fill
        cap = FIRST_CAP if not chunks else CHUNK_MAX
        if cur_len and cur_len + slot_len[j] > cap:
            chunks.append((cur0, j - cur0, slot_off[cur0], cur_len))
            cur0, cur_len = j, 0
        cur_len += slot_len[j]
    if cur_len:
        chunks.append((cur0, nslot - cur0, slot_off[cur0], cur_len))
    # split a small tail off the last chunk so the pipeline drains fast
    (s0, ns, coff, clen) = chunks[-1]
    if ns > 2 and clen > 2 * LAST_CAP:
        cut, cut_len = ns, 0
        while cut > 1 and cut_len + slot_len[s0 + cut - 1] <= LAST_CAP:
            cut -= 1
            cut_len += slot_len[s0 + cut]
        if 0 < cut < ns:
            chunks[-1] = (s0, cut, coff, clen - cut_len)
            chunks.append((s0 + cut, ns - cut, slot_off[s0 + cut], cut_len))
    return cnt, starts, percore, slot_len, slot_off, chunks, T


def _build(slot_len, slot_off, chunks, T):
    nslot = len(slot_len)
    A = mybir.AluOpType
    nc = bacc.Bacc()
    xcm = nc.dram_tensor("xcm", [128, T], f16, kind="ExternalInput")
    invr = nc.dram_tensor("invr", [128, 2 * nslot], f32, kind="ExternalInput")
    pb = nc.dram_tensor("pb", [128, 4], f32, kind="ExternalInput")
    outp = nc.dram_tensor("outp", [128, T], f16, kind="ExternalOutput")

    nchunk = len(chunks)

    with tile.TileContext(nc) as tc:
        with tc.tile_pool(name="const", bufs=1) as constp, \
             tc.tile_pool(name="dpool", bufs=PREFETCH + 1) as dpool, \
             tc.tile_pool(name="opool", bufs=3) as opool, \
             tc.tile_pool(name="scrp", bufs=1) as scrp, \
             tc.tile_pool(name="statp", bufs=3) as statp:

            invt = constp.tile([128, 2 * nslot], f32)
            nc.sync.dma_start(out=invt, in_=invr.ap()[:, :])
            pbt = constp.tile([128, 4], f32)
            nc.sync.dma_start(out=pbt, in_=pb.ap()[:, :])
            epst = constp.tile([128, 1], f32)
            nc.vector.memset(epst, EPS)
            w_col = pbt[:, 0:1]
            b_col = pbt[:, 1:2]
            negs_col = pbt[:, 2:3]
            coef_col = pbt[:, 3:4]

            SCR = scrp.tile([128, CHUNK_MAX], f16)

            Dt = [None] * nchunk

            def load(c):
                (s0, ns, coff, clen) = chunks[c]
                D = dpool.tile([128, CHUNK_MAX], f16, tag="D")
                nc.sync.dma_start(out=D[:, 0:clen],
                                  in_=xcm.ap()[:, coff:coff + clen])
                Dt[c] = D

            for c in range(min(PREFETCH + 1, nchunk)):
                load(c)

            for c, (s0, ns, coff, clen) in enumerate(chunks):
                D = Dt[c]
                OUT = opool.tile([128, CHUNK_MAX], f16, tag="OUT")
                sums = statp.tile([128, 2 * ns], f32, tag="sums")
                sumx = sums[:, 0:ns]
                sumx2 = sums[:, ns:2 * ns]

                # ---- sum(x): per equal-length slot group, fold tree on DVE
                i = 0
                while i < ns:
                    L = slot_len[s0 + i]
                    j = i
                    while j < ns and slot_len[s0 + j] == L:
                        j += 1
                    gs = j - i  # group size
                    a = slot_off[s0 + i] - coff
                    src = D[:, a:a + gs * L].rearrange("p (s l) -> p s l", l=L)
                    scr_off = 0
                    Lc = L
                    while Lc > FOLD_MIN and Lc % 2 == 0:
                        h = Lc // 2
                        dst = SCR[:, scr_off:scr_off + gs * h].rearrange(
                            "p (s l) -> p s l", l=h)
                        nc.vector.tensor_tensor(
                            out=dst, in0=src[:, :, 0:h], in1=src[:, :, h:Lc],
                            op=A.add)
                        src = dst
                        scr_off += gs * h
                        Lc = h
                    nc.vector.tensor_reduce(
                        out=sumx[:, i:j], in_=src,
                        axis=mybir.AxisListType.X, op=A.add)
                    i = j

                # ---- sum(x^2): ACT Square + accumulate, per slot
                for i in range(ns):
                    a = slot_off[s0 + i] - coff
                    e = a + slot_len[s0 + i]
                    nc.scalar.activation(
                        out=OUT[:, a:e], in_=D[:, a:e],
                        func=mybir.ActivationFunctionType.Square,
                        accum_out=sumx2[:, i:i + 1])

                # ---- stats -> per-slot affine maps W, B ([128, ns] f32)
                st = statp.tile([128, 4 * ns], f32, tag="st")
                mom = st[:, 0:2 * ns]       # [mean | ex2]
                mean = st[:, 0:ns]
                ex2 = st[:, ns:2 * ns]
                t2 = st[:, 2 * ns:3 * ns]
                istd = st[:, 3 * ns:4 * ns]
                wb = statp.tile([128, 2 * ns], f32, tag="wb")
                Wt = wb[:, 0:ns]
                Bt = wb[:, ns:2 * ns]

                # [mean|ex2] = [sumx|sumx2] * [inv|inv] (invr holds inv twice)
                inv2 = invt.rearrange("p (h n) -> p h n",
                                      n=nslot)[:, :, s0:s0 + ns]
                nc.vector.tensor_tensor(
                    out=mom.rearrange("p (h n) -> p h n", n=ns),
                    in0=sums.rearrange("p (h n) -> p h n", n=ns),
                    in1=inv2, op=A.mult)
                nc.vector.tensor_tensor(out=t2, in0=mean, in1=mean, op=A.mult)
                nc.vector.tensor_scalar(out=t2, in0=t2, scalar1=coef_col,
                                        scalar2=None, op0=A.mult)
                nc.vector.tensor_tensor(out=t2, in0=ex2, in1=t2,
                                        op=A.subtract)
                nc.scalar.activation(out=t2, in_=t2,
                                     func=mybir.ActivationFunctionType.Sqrt,
                                     bias=epst)
                nc.vector.reciprocal(out=istd, in_=t2)
                nc.vector.tensor_scalar(out=Wt, in0=istd, scalar1=w_col,
                                        scalar2=None, op0=A.mult)
                nc.vector.tensor_tensor(out=t2, in0=mean, in1=Wt, op=A.mult)
                nc.vector.tensor_scalar(out=Bt, in0=t2, scalar1=negs_col,
                                        scalar2=b_col, op0=A.mult, op1=A.add)

                # ---- fused apply: out = x*W[slot] + B[slot]
                for i in range(ns):
                    a = slot_off[s0 + i] - coff
                    e = a + slot_len[s0 + i]
                    nc.vector.tensor_scalar(
                        out=OUT[:, a:e], in0=D[:, a:e],
                        scalar1=Wt[:, i:i + 1], scalar2=Bt[:, i:i + 1],
                        op0=A.mult, op1=A.add)
                nc.sync.dma_start(out=outp.ap()[:, coff:coff + clen],
                                  in_=OUT[:, 0:clen])
                if c + PREFETCH + 1 < nchunk:
                    load(c + PREFETCH + 1)

    nc.finalize()
    return nc


def kernel(x, batch, weight, bias, mean_scale, batch_size):
    x = np.asarray(x, dtype=np.float32)
    batch_np = np.asarray(batch).astype(np.int64)
    w = np.asarray(weight, dtype=np.float32)
    b = np.asarray(bias, dtype=np.float32)
    s = np.asarray(mean_scale, dtype=np.float32)
    assert x.shape == (N, C) and int(batch_size) == B

    cnt, starts, percore, slot_len, slot_off, chunks, T = _plan(batch_np)
    nslot = len(slot_len)

    key = tuple(slot_len)
    if key not in _prog_cache:
        _prog_cache[key] = _build(slot_len, slot_off, chunks, T)
    nc = _prog_cache[key]

    x16 = x.astype(np.float16)
    pbm = np.ascontiguousarray(
        np.stack([w, b, -s, 2.0 * s - s * s], axis=1), dtype=np.float32)

    in_maps = []
    for k in range(NCORES):
        xb = np.zeros((T, C), np.float16)
        invm = np.zeros((2 * nslot,), np.float32)
        for j, g in enumerate(percore[k]):
            a = int(starts[g])
            n = int(cnt[g])
            o = slot_off[j]
            xb[o:o + n] = x16[a:a + n]
            invm[j] = 1.0 / n
            invm[nslot + j] = 1.0 / n
        xcm_np = np.ascontiguousarray(xb.T)
        inv128 = np.ascontiguousarray(
            np.broadcast_to(invm[None, :], (128, 2 * nslot)), dtype=np.float32)
        in_maps.append({"xcm": xcm_np, "invr": inv128, "pb": pbm})

    import os
    kw = {}
    if os.environ.get("GN_TRACE", "0") == "1":
        kw = {"trace": True,
              "tmpdir": os.environ.get("GN_TRACE_DIR") or None}
    res = run_bass_kernel_spmd(nc, in_maps, core_ids=list(range(NCORES)), **kw)
    global last_results
    last_results = res

    out = np.empty((N, C), np.float32)
    for k in range(NCORES):
        op = np.asarray(res.results[k]["outp"])  # [128, T] f16
        opT = np.ascontiguousarray(op.T)
        for j, g in enumerate(percore[k]):
            a = int(starts[g])
            n = int(cnt[g])
            o = slot_off[j]
            out[a:a + n] = opT[o:o + n]
    return out
